# revision 26
# baseline (speedup 1.0000x reference)
"""LocalAttention (banded) Trainium2 kernel, 8-core SPMD.

Problem: B=2, S=2048, H=1024, nh=16, hd=64, window=256 (half_w=128).
  q = x@Wq+bq ; k = x@Wk+bk ; v = x@Wv+bv  (per-head dim 64)
  scores = q.k/8 masked to |i-j|<=128 ; out = softmax(scores)@v @ Wo + bo

Sharding: core c -> batch c//4, token block (c%4)*512..+512.  Each core
receives a zero-padded 768-token slice of x (128-token halo each side,
recomputed locally; no cross-core communication).

On-chip layout is fully "transposed": features on partitions, tokens on
the free dim.  Host passes x^T (bf16).  Scores are computed transposed
(S^T = K @ Q^T) so the PV matmul can contract over the key dim.  Each
per-head V block carries 64 ones columns (flash-attention style): the PV
matmul emits the softmax denominator replicated across output partitions
64..127, where a fast approximate reciprocal + multiply normalizes it.
All matmul operands bf16, PSUM accumulation fp32.  1/sqrt(hd) is folded
into Wq/bq and bv@Wo+bo into a single output bias on the host.

The instruction stream is software-pipelined so the PE array rarely
waits on Scalar(exp)/Vector work: Q projections first (weights stream
from HBM over three DMA queues), then V / K / score matmuls interleaved,
then PV with lagged exp dependencies, then the output projection.  A
short warm-up matmul burst ramps the PE p-state while the first weight
tiles land.
"""

import sys

if "/opt/trn_rl_repo" not in sys.path:
    sys.path.insert(0, "/opt/trn_rl_repo")

import numpy as np
import ml_dtypes

B, S, H = 2, 2048, 1024
NH, HD = 16, 64
HALF_W = 128
NCORES = 8
BLK = 512          # owned tokens per core
PAD = 768          # owned + 2*128 halo
NQB = 4            # q-blocks of 128 per core
NKC = 6            # padded-local k chunks of 128
VOW = 128 * 16     # V+ones tile width: 16 head-subs x (64 feats + 64 ones)
BF16 = ml_dtypes.bfloat16

_COMPILED = None


def _build_core_inputs(x, Wq, bq, Wk, bk, Wv, bv, Wo, bo):
    """Host-side sharding / layout prep. Returns list of 8 in_maps."""
    x = np.asarray(x, np.float32)
    scale = 1.0 / np.sqrt(HD)
    wq_s = (np.asarray(Wq, np.float32) * scale).astype(BF16)
    wk_s = np.asarray(Wk, np.float32).astype(BF16)
    wv_s = np.asarray(Wv, np.float32).astype(BF16)
    wo_s = np.asarray(Wo, np.float32).astype(BF16)
    bq_s = (np.asarray(bq, np.float32) * scale)
    bk_s = np.asarray(bk, np.float32)
    # v-bias passes through attention unchanged (softmax rows sum to 1),
    # so it folds into the output bias: bo' = bo + bv @ Wo.
    bo_s = np.asarray(bo, np.float32) + np.asarray(bv, np.float32) @ np.asarray(Wo, np.float32)

    def as_pcols(vec):  # [1024] -> [128, 8] with [:, c] = vec[128c:128c+128]
        return np.ascontiguousarray(vec.reshape(8, 128).T, dtype=np.float32)

    bq_t, bk_t, bo_t = as_pcols(bq_s), as_pcols(bk_s), as_pcols(bo_s)

    in_maps = []
    for c in range(NCORES):
        b, blk = divmod(c, 4)
        t0 = blk * BLK
        lo, hi = t0 - HALF_W, t0 + BLK + HALF_W
        xp = np.zeros((PAD, H), np.float32)
        glo, ghi = max(lo, 0), min(hi, S)
        xp[glo - lo:ghi - lo] = x[b, glo:ghi]
        xT = np.ascontiguousarray(xp.T, dtype=BF16)  # [1024, 768]

        # Mask tiles match the transposed expS layout: for q-block qb and
        # relative key chunk rel, tile element [p, qb*384 + 128*rel + i]
        # guards key token lo+128*qb+128*rel+p vs query token t0+128*qb+i.
        mask = np.zeros((128, NQB * 384), BF16)
        for qb in range(NQB):
            qg = t0 + 128 * qb + np.arange(128)          # query token per free col
            for rel in range(3):
                kg = lo + 128 * (qb + rel) + np.arange(128)  # key token per partition
                valid = (np.abs(kg[:, None] - qg[None, :]) <= HALF_W) & \
                        (kg[:, None] >= 0) & (kg[:, None] < S)
                mask[:, qb * 384 + 128 * rel: qb * 384 + 128 * (rel + 1)] = valid
        in_maps.append({
            "xT": xT,
            "wq": wq_s, "wk": wk_s, "wv": wv_s, "wo": wo_s,
            "bq_t": bq_t, "bk_t": bk_t, "bo_t": bo_t,
            "mask": mask,
        })
    return in_maps


def _build_bass():
    import concourse.bass as bass
    import concourse.tile as tile
    from concourse import bacc, mybir
    from contextlib import ExitStack

    f32, bf16 = mybir.dt.float32, mybir.dt.bfloat16
    Id = mybir.ActivationFunctionType.Identity
    Exp = mybir.ActivationFunctionType.Exp

    nc = bacc.Bacc(None)
    d_xT = nc.declare_dram_parameter("xT", [H, PAD], bf16, isOutput=False)
    d_wq = nc.declare_dram_parameter("wq", [H, H], bf16, isOutput=False)
    d_wk = nc.declare_dram_parameter("wk", [H, H], bf16, isOutput=False)
    d_wv = nc.declare_dram_parameter("wv", [H, H], bf16, isOutput=False)
    d_wo = nc.declare_dram_parameter("wo", [H, H], bf16, isOutput=False)
    d_bq = nc.declare_dram_parameter("bq_t", [128, 8], f32, isOutput=False)
    d_bk = nc.declare_dram_parameter("bk_t", [128, 8], f32, isOutput=False)
    d_bo = nc.declare_dram_parameter("bo_t", [128, 8], f32, isOutput=False)
    d_mask = nc.declare_dram_parameter("mask", [128, NQB * 384], bf16, isOutput=False)
    d_out = nc.declare_dram_parameter("out", [H, BLK], bf16, isOutput=True)

    with tile.TileContext(nc) as tc, ExitStack() as ctx:
        # ---- long-lived tiles -------------------------------------------
        persist = ctx.enter_context(tc.tile_pool(name="persist", bufs=1))
        sb_xT = [persist.tile([128, PAD], bf16, name=f"xT{h}", tag=f"xT{h}") for h in range(8)]
        sb_wq = [persist.tile([128, H], bf16, name=f"wq{h}", tag=f"wq{h}") for h in range(8)]
        sb_wk = [persist.tile([128, H], bf16, name=f"wk{h}", tag=f"wk{h}") for h in range(8)]
        sb_wv = [persist.tile([128, H], bf16, name=f"wv{h}", tag=f"wv{h}") for h in range(8)]
        sb_wo = [persist.tile([128, H], bf16, name=f"wo{h}", tag=f"wo{h}") for h in range(8)]
        sb_mask = persist.tile([128, NQB * 384], bf16, name="mask", tag="mask")
        sb_bq = persist.tile([128, 8], f32, name="bq", tag="bq")
        sb_bk = persist.tile([128, 8], f32, name="bk", tag="bk")
        sb_bo = persist.tile([128, 8], f32, name="bo", tag="bo")
        sb_qt = [persist.tile([128, BLK], bf16, name=f"qt{c}", tag=f"qt{c}") for c in range(8)]
        sb_kt = [persist.tile([128, PAD], bf16, name=f"kt{c}", tag=f"kt{c}") for c in range(8)]
        # V with 64 ones columns per head-sub: slot j covers cols 128j..128j+128,
        # cols 128j+64.. are 1.0 so the PV matmul emits the softmax denominator
        # replicated across output partitions 64..127 (broadcast for free).
        sb_vo = [persist.tile([128, VOW], bf16, name=f"vo{t}", tag=f"vo{t}") for t in range(NKC)]
        sb_oc = [persist.tile([128, BLK], bf16, name=f"oc{c}", tag=f"oc{c}") for c in range(8)]
        sb_warm = persist.tile([128, 128], bf16, name="warm", tag="warm")

        # DMA issue order doubles as priority; three hardware queues run in
        # parallel (sync / scalar-act / gpsimd).
        for h in range(4):
            nc.sync.dma_start(sb_xT[h][:], d_xT[128 * h:128 * (h + 1), :])
            nc.scalar.dma_start(sb_xT[h + 4][:], d_xT[128 * (h + 4):128 * (h + 5), :])
        for h in range(8):
            nc.gpsimd.dma_start(sb_wq[h][:], d_wq[128 * h:128 * (h + 1), :])
        nc.scalar.dma_start(sb_bq[:], d_bq[:])
        nc.scalar.dma_start(sb_bk[:], d_bk[:])
        for h in range(8):
            nc.gpsimd.dma_start(sb_wv[h][:], d_wv[128 * h:128 * (h + 1), :])
        for h in range(8):
            nc.gpsimd.dma_start(sb_wk[h][:], d_wk[128 * h:128 * (h + 1), :])
        nc.scalar.dma_start(sb_mask[:], d_mask[:])
        for h in range(8):
            nc.sync.dma_start(sb_wo[h][:], d_wo[128 * h:128 * (h + 1), :])
        nc.scalar.dma_start(sb_bo[:], d_bo[:])

        nc.vector.memset(sb_warm[:], 1.0)
        for t in range(NKC):
            v_ap = sb_vo[t][:]
            ones_cols = bass.AP(tensor=v_ap.tensor, offset=v_ap.offset + 64,
                                ap=[v_ap.ap[0], [128, 16], [1, 64]])
            nc.vector.memset(ones_cols, 1.0)

        with tc.tile_pool(name="ps_a", bufs=2, space="PSUM") as ps_a, \
             tc.tile_pool(name="ps_b", bufs=3, space="PSUM") as ps_b, \
             tc.tile_pool(name="ps_po", bufs=3, space="PSUM") as ps_po, \
             tc.tile_pool(name="att", bufs=26) as att, \
             tc.tile_pool(name="rbp", bufs=3) as rbp, \
             tc.tile_pool(name="yout", bufs=3) as yout:

            exps = {}          # (c, qb) -> expS^T pair tile [128 k, 768 = 2 heads x 384]

            # ramp the PE p-state while input DMAs land: the tensor engine
            # needs ~3us of continuous execution to reach max clock.
            pwarm = ps_po.tile([128, 32], f32, name="pwarm", tag="po", padded_shape=[128, BLK])
            for _ in range(20):
                nc.tensor.matmul(pwarm[:], sb_warm[:], sb_warm[:, :32])

            def do_q(c):
                ps = ps_a.tile([128, BLK], f32, name="psq", tag="psa")
                for h in range(8):
                    nc.tensor.matmul(ps[:], sb_wq[h][:, 128 * c:128 * (c + 1)],
                                     sb_xT[h][:, 128:128 + BLK],
                                     start=(h == 0), stop=(h == 7))
                nc.vector.tensor_scalar_add(sb_qt[c][:], ps[:], sb_bq[:, c:c + 1])

            def do_k(c):
                for half in range(2):
                    ps = ps_b.tile([128, 384], f32, name="psk", tag="psb")
                    for h in range(8):
                        nc.tensor.matmul(ps[:], sb_wk[h][:, 128 * c:128 * (c + 1)],
                                         sb_xT[h][:, 384 * half:384 * (half + 1)],
                                         start=(h == 0), stop=(h == 7))
                    nc.vector.tensor_scalar_add(
                        sb_kt[c][:, 384 * half:384 * (half + 1)], ps[:],
                        sb_bk[:, c:c + 1])

            def do_v(t):
                # token-major V: stationary = xT chunk, moving = Wv rows.
                # Copy per-head-sub 64-col chunks into the 128-strided vo slots.
                for half in range(2):
                    ps = ps_a.tile([128, BLK], f32, name="psv", tag="psa")
                    for h in range(8):
                        nc.tensor.matmul(ps[:], sb_xT[h][:, 128 * t:128 * (t + 1)],
                                         sb_wv[h][:, 512 * half:512 * (half + 1)],
                                         start=(h == 0), stop=(h == 7))
                    s_ap, v_ap = ps[:], sb_vo[t][:]
                    src = bass.AP(tensor=s_ap.tensor, offset=s_ap.offset,
                                  ap=[s_ap.ap[0], [64, 8], [1, 64]])
                    dst = bass.AP(tensor=v_ap.tensor, offset=v_ap.offset + 128 * 8 * half,
                                  ap=[v_ap.ap[0], [128, 8], [1, 64]])
                    nc.vector.tensor_copy(dst, src)

            def do_scores(c, qbs=range(NQB)):
                # scores + exp + mask for heads 2c (sub=0), 2c+1 (sub=1)
                for qb in qbs:
                    e = att.tile([128, 768], bf16, name="exps", tag="exps")
                    for sub in range(2):
                        ps = ps_b.tile([128, 384], f32, name="pss", tag="psb")
                        for rel in range(3):
                            kc = qb + rel
                            nc.tensor.matmul(
                                ps[:, 128 * rel:128 * (rel + 1)],
                                sb_kt[c][64 * sub:64 * (sub + 1), 128 * kc:128 * (kc + 1)],
                                sb_qt[c][64 * sub:64 * (sub + 1), 128 * qb:128 * (qb + 1)])
                        nc.scalar.activation(e[:, 384 * sub:384 * (sub + 1)], ps[:], Exp)
                    # mask both heads, off-diagonal chunks only (rel=1 is
                    # always fully valid): strided per-chunk views
                    m = sb_mask[:, 384 * qb:384 * (qb + 1)]
                    for rel in (0, 2):
                        mv = bass.AP(tensor=m.tensor, offset=m.offset + 128 * rel,
                                     ap=[m.ap[0], [0, 2], [1, 128]])
                        ev = bass.AP(tensor=e.tensor, offset=e[:].offset + 128 * rel,
                                     ap=[e[:].ap[0], [384, 2], [1, 128]])
                        nc.gpsimd.tensor_mul(ev, ev, mv)
                    exps[(c, qb)] = e

            def do_pv(c):
                for sub in range(2):
                    j = 2 * c + sub
                    # after S7, ps_b is idle: borrow it for the tail pairs'
                    # second accumulator to deepen the PSUM ring.
                    if c >= 5 and sub == 1:
                        po = ps_b.tile([128, BLK], f32, name="po2", tag="psb")
                    else:
                        po = ps_po.tile([128, BLK], f32, name="po", tag="po")
                    for qb in range(NQB):
                        for rel in range(3):
                            kc = qb + rel
                            nc.tensor.matmul(
                                po[:, 128 * qb:128 * (qb + 1)],
                                sb_vo[kc][:, 128 * j:128 * (j + 1)],
                                exps[(c, qb)][:, 384 * sub + 128 * rel:384 * sub + 128 * (rel + 1)],
                                start=(rel == 0), stop=(rel == 2))
                    # denominator is replicated on partitions 64..127; the
                    # approx-fast reciprocal needs an SBUF fp32 source, so
                    # stage it through the scalar engine (denom >= 1: safe).
                    pr = rbp.tile([64, BLK], f32, name="pr", tag="pr")
                    nc.scalar.copy(pr[:], po[64:128, :])
                    rb = rbp.tile([64, BLK], f32, name="rb", tag="rb")
                    nc.vector.reciprocal_approx_fast(out=rb[:], in_=pr[:])
                    nc.vector.tensor_mul(sb_oc[c][64 * sub:64 * (sub + 1), :],
                                         po[0:64, :], rb[:])
                for qb in range(NQB):
                    del exps[(c, qb)]

            # ---- software-pipelined issue order -------------------------
            for c in range(8):
                do_q(c)
            do_v(0); do_v(1); do_k(0)
            do_v(2); do_k(1); do_scores(0)
            do_v(3); do_k(2); do_scores(1)
            do_v(4); do_k(3); do_scores(2)
            do_v(5); do_k(4); do_scores(3)
            do_k(5); do_scores(4)
            do_k(6); do_pv(0); do_scores(5)
            do_k(7); do_pv(1); do_scores(6, (0, 1))
            do_pv(2); do_scores(6, (2, 3))
            do_pv(3); do_scores(7, (0, 1))
            do_pv(4); do_scores(7, (2, 3))
            do_pv(5); do_pv(6)

            # open the first two output blocks' accumulation before the last
            # PV so the final normalize chains hide behind matmuls.
            pre = {}
            for c0 in (0, 1):
                pre[c0] = ps_a.tile([128, BLK], f32, name="psy", tag="psa")
                for f in range(6):
                    nc.tensor.matmul(pre[c0][:], sb_wo[f][:, 128 * c0:128 * (c0 + 1)],
                                     sb_oc[f][:], start=(f == 0), stop=False)
            do_pv(7)

            # ---- output projection --------------------------------------
            for c in range(8):
                ps = pre[c] if c in pre else ps_a.tile([128, BLK], f32, name="psy", tag="psa")
                for f in range(6 if c in pre else 0, 8):
                    nc.tensor.matmul(ps[:], sb_wo[f][:, 128 * c:128 * (c + 1)],
                                     sb_oc[f][:], start=(f == 0), stop=(f == 7))
                yt = yout.tile([128, BLK], bf16, name="yt", tag="yt")
                nc.scalar.activation(yt[:], ps[:], Id, bias=sb_bo[:, c:c + 1])
                nc.sync.dma_start(d_out[128 * c:128 * (c + 1), :], yt[:])

    nc.compile()
    return nc


def _get_compiled():
    global _COMPILED
    if _COMPILED is None:
        _COMPILED = _build_bass()
    return _COMPILED


def kernel(x, Wq, bq, Wk, bk, Wv, bv, Wo, bo, _trace=False):
    from concourse.bass_utils import run_bass_kernel_spmd

    in_maps = _build_core_inputs(x, Wq, bq, Wk, bk, Wv, bv, Wo, bo)
    nc = _get_compiled()
    res = run_bass_kernel_spmd(nc, in_maps, core_ids=list(range(NCORES)),
                               trace=_trace)
    out = np.empty((B, S, H), np.float32)
    for c in range(NCORES):
        b, blk = divmod(c, 4)
        out[b, blk * BLK:(blk + 1) * BLK, :] = res.results[c]["out"].T.astype(np.float32)
    if _trace:
        return out, res
    return out


# revision 27
# speedup vs baseline: 1.0134x; 1.0134x over previous
"""LocalAttention (banded) Trainium2 kernel, 8-core SPMD.

Problem: B=2, S=2048, H=1024, nh=16, hd=64, window=256 (half_w=128).
  q = x@Wq+bq ; k = x@Wk+bk ; v = x@Wv+bv  (per-head dim 64)
  scores = q.k/8 masked to |i-j|<=128 ; out = softmax(scores)@v @ Wo + bo

Sharding: core c -> batch c//4, token block (c%4)*512..+512.  Each core
receives a zero-padded 768-token slice of x (128-token halo each side,
recomputed locally; no cross-core communication).

On-chip layout is fully "transposed": features on partitions, tokens on
the free dim.  Host passes x^T (bf16).  Scores are computed transposed
(S^T = K @ Q^T) so the PV matmul can contract over the key dim.  Each
per-head V block carries 64 ones columns (flash-attention style): the PV
matmul emits the softmax denominator replicated across output partitions
64..127, where a fast approximate reciprocal + multiply normalizes it.
All matmul operands bf16, PSUM accumulation fp32.  1/sqrt(hd) is folded
into Wq/bq and bv@Wo+bo into a single output bias on the host.

The instruction stream is software-pipelined so the PE array rarely
waits on Scalar(exp)/Vector work: Q projections first (weights stream
from HBM over three DMA queues), then V / K / score matmuls interleaved,
then PV with lagged exp dependencies, then the output projection.  A
short warm-up matmul burst ramps the PE p-state while the first weight
tiles land.
"""

import sys

if "/opt/trn_rl_repo" not in sys.path:
    sys.path.insert(0, "/opt/trn_rl_repo")

import numpy as np
import ml_dtypes

B, S, H = 2, 2048, 1024
NH, HD = 16, 64
HALF_W = 128
NCORES = 8
BLK = 512          # owned tokens per core
PAD = 768          # owned + 2*128 halo
NQB = 4            # q-blocks of 128 per core
NKC = 6            # padded-local k chunks of 128
VOW = 128 * 16     # V+ones tile width: 16 head-subs x (64 feats + 64 ones)
BF16 = ml_dtypes.bfloat16

_COMPILED = None


def _build_core_inputs(x, Wq, bq, Wk, bk, Wv, bv, Wo, bo):
    """Host-side sharding / layout prep. Returns list of 8 in_maps."""
    x = np.asarray(x, np.float32)
    scale = 1.0 / np.sqrt(HD)
    wq_s = (np.asarray(Wq, np.float32) * scale).astype(BF16)
    wk_s = np.asarray(Wk, np.float32).astype(BF16)
    wv_s = np.asarray(Wv, np.float32).astype(BF16)
    wo_s = np.asarray(Wo, np.float32).astype(BF16)
    bq_s = (np.asarray(bq, np.float32) * scale)
    bk_s = np.asarray(bk, np.float32)
    # v-bias passes through attention unchanged (softmax rows sum to 1),
    # so it folds into the output bias: bo' = bo + bv @ Wo.
    bo_s = np.asarray(bo, np.float32) + np.asarray(bv, np.float32) @ np.asarray(Wo, np.float32)

    def as_pcols(vec):  # [1024] -> [128, 8] with [:, c] = vec[128c:128c+128]
        return np.ascontiguousarray(vec.reshape(8, 128).T, dtype=np.float32)

    bq_t, bk_t, bo_t = as_pcols(bq_s), as_pcols(bk_s), as_pcols(bo_s)

    in_maps = []
    for c in range(NCORES):
        b, blk = divmod(c, 4)
        t0 = blk * BLK
        lo, hi = t0 - HALF_W, t0 + BLK + HALF_W
        xp = np.zeros((PAD, H), np.float32)
        glo, ghi = max(lo, 0), min(hi, S)
        xp[glo - lo:ghi - lo] = x[b, glo:ghi]
        xT = np.ascontiguousarray(xp.T, dtype=BF16)  # [1024, 768]

        # Mask tiles match the transposed expS layout: for q-block qb and
        # relative key chunk rel, tile element [p, qb*384 + 128*rel + i]
        # guards key token lo+128*qb+128*rel+p vs query token t0+128*qb+i.
        mask = np.zeros((128, NQB * 384), BF16)
        for qb in range(NQB):
            qg = t0 + 128 * qb + np.arange(128)          # query token per free col
            for rel in range(3):
                kg = lo + 128 * (qb + rel) + np.arange(128)  # key token per partition
                valid = (np.abs(kg[:, None] - qg[None, :]) <= HALF_W) & \
                        (kg[:, None] >= 0) & (kg[:, None] < S)
                mask[:, qb * 384 + 128 * rel: qb * 384 + 128 * (rel + 1)] = valid
        in_maps.append({
            "xT": xT,
            "wq": wq_s, "wk": wk_s, "wv": wv_s, "wo": wo_s,
            "bq_t": bq_t, "bk_t": bk_t, "bo_t": bo_t,
            "mask": mask,
        })
    return in_maps


def _build_bass():
    import concourse.bass as bass
    import concourse.tile as tile
    from concourse import bacc, mybir
    from contextlib import ExitStack

    f32, bf16 = mybir.dt.float32, mybir.dt.bfloat16
    Id = mybir.ActivationFunctionType.Identity
    Exp = mybir.ActivationFunctionType.Exp

    nc = bacc.Bacc(None)
    d_xT = nc.declare_dram_parameter("xT", [H, PAD], bf16, isOutput=False)
    d_wq = nc.declare_dram_parameter("wq", [H, H], bf16, isOutput=False)
    d_wk = nc.declare_dram_parameter("wk", [H, H], bf16, isOutput=False)
    d_wv = nc.declare_dram_parameter("wv", [H, H], bf16, isOutput=False)
    d_wo = nc.declare_dram_parameter("wo", [H, H], bf16, isOutput=False)
    d_bq = nc.declare_dram_parameter("bq_t", [128, 8], f32, isOutput=False)
    d_bk = nc.declare_dram_parameter("bk_t", [128, 8], f32, isOutput=False)
    d_bo = nc.declare_dram_parameter("bo_t", [128, 8], f32, isOutput=False)
    d_mask = nc.declare_dram_parameter("mask", [128, NQB * 384], bf16, isOutput=False)
    d_out = nc.declare_dram_parameter("out", [H, BLK], bf16, isOutput=True)

    with tile.TileContext(nc) as tc, ExitStack() as ctx:
        # ---- long-lived tiles -------------------------------------------
        persist = ctx.enter_context(tc.tile_pool(name="persist", bufs=1))
        sb_xT = [persist.tile([128, PAD], bf16, name=f"xT{h}", tag=f"xT{h}") for h in range(8)]
        sb_wq = [persist.tile([128, H], bf16, name=f"wq{h}", tag=f"wq{h}") for h in range(8)]
        sb_wk = [persist.tile([128, H], bf16, name=f"wk{h}", tag=f"wk{h}") for h in range(8)]
        sb_wv = [persist.tile([128, H], bf16, name=f"wv{h}", tag=f"wv{h}") for h in range(8)]
        sb_wo = [persist.tile([128, H], bf16, name=f"wo{h}", tag=f"wo{h}") for h in range(8)]
        sb_mask = persist.tile([128, NQB * 384], bf16, name="mask", tag="mask")
        sb_bq = persist.tile([128, 8], f32, name="bq", tag="bq")
        sb_bk = persist.tile([128, 8], f32, name="bk", tag="bk")
        sb_bo = persist.tile([128, 8], f32, name="bo", tag="bo")
        sb_qt = [persist.tile([128, BLK], bf16, name=f"qt{c}", tag=f"qt{c}") for c in range(8)]
        sb_kt = [persist.tile([128, PAD], bf16, name=f"kt{c}", tag=f"kt{c}") for c in range(8)]
        # V with 64 ones columns per head-sub: slot j covers cols 128j..128j+128,
        # cols 128j+64.. are 1.0 so the PV matmul emits the softmax denominator
        # replicated across output partitions 64..127 (broadcast for free).
        sb_vo = [persist.tile([128, VOW], bf16, name=f"vo{t}", tag=f"vo{t}") for t in range(NKC)]
        sb_oc = [persist.tile([128, BLK], bf16, name=f"oc{c}", tag=f"oc{c}") for c in range(8)]
        sb_warm = persist.tile([128, 128], bf16, name="warm", tag="warm")

        # DMA issue order doubles as priority; three hardware queues run in
        # parallel (sync / scalar-act / gpsimd).
        for h in range(4):
            nc.sync.dma_start(sb_xT[h][:], d_xT[128 * h:128 * (h + 1), :])
            nc.scalar.dma_start(sb_xT[h + 4][:], d_xT[128 * (h + 4):128 * (h + 5), :])
        for h in range(8):
            nc.gpsimd.dma_start(sb_wq[h][:], d_wq[128 * h:128 * (h + 1), :])
        nc.scalar.dma_start(sb_bq[:], d_bq[:])
        nc.scalar.dma_start(sb_bk[:], d_bk[:])
        for h in range(8):
            nc.gpsimd.dma_start(sb_wv[h][:], d_wv[128 * h:128 * (h + 1), :])
        for h in range(8):
            nc.gpsimd.dma_start(sb_wk[h][:], d_wk[128 * h:128 * (h + 1), :])
        nc.scalar.dma_start(sb_mask[:], d_mask[:])
        for h in range(8):
            nc.sync.dma_start(sb_wo[h][:], d_wo[128 * h:128 * (h + 1), :])
        nc.scalar.dma_start(sb_bo[:], d_bo[:])

        nc.vector.memset(sb_warm[:], 1.0)
        for t in range(NKC):
            v_ap = sb_vo[t][:]
            ones_cols = bass.AP(tensor=v_ap.tensor, offset=v_ap.offset + 64,
                                ap=[v_ap.ap[0], [128, 16], [1, 64]])
            nc.vector.memset(ones_cols, 1.0)

        with tc.tile_pool(name="ps_a", bufs=2, space="PSUM") as ps_a, \
             tc.tile_pool(name="ps_b", bufs=3, space="PSUM") as ps_b, \
             tc.tile_pool(name="ps_po", bufs=3, space="PSUM") as ps_po, \
             tc.tile_pool(name="att", bufs=26) as att, \
             tc.tile_pool(name="rbp", bufs=3) as rbp, \
             tc.tile_pool(name="yout", bufs=3) as yout:

            exps = {}          # (c, qb) -> expS^T pair tile [128 k, 768 = 2 heads x 384]

            # ramp the PE p-state while input DMAs land: the tensor engine
            # needs ~3us of continuous execution to reach max clock.
            pwarm = ps_po.tile([128, 32], f32, name="pwarm", tag="po", padded_shape=[128, BLK])
            for _ in range(20):
                nc.tensor.matmul(pwarm[:], sb_warm[:], sb_warm[:, :32])

            def do_q(c):
                ps = ps_a.tile([128, BLK], f32, name="psq", tag="psa")
                for h in range(8):
                    nc.tensor.matmul(ps[:], sb_wq[h][:, 128 * c:128 * (c + 1)],
                                     sb_xT[h][:, 128:128 + BLK],
                                     start=(h == 0), stop=(h == 7))
                nc.vector.tensor_scalar_add(sb_qt[c][:], ps[:], sb_bq[:, c:c + 1])

            def do_k(c):
                for half in range(2):
                    ps = ps_b.tile([128, 384], f32, name="psk", tag="psb")
                    for h in range(8):
                        nc.tensor.matmul(ps[:], sb_wk[h][:, 128 * c:128 * (c + 1)],
                                         sb_xT[h][:, 384 * half:384 * (half + 1)],
                                         start=(h == 0), stop=(h == 7))
                    nc.vector.tensor_scalar_add(
                        sb_kt[c][:, 384 * half:384 * (half + 1)], ps[:],
                        sb_bk[:, c:c + 1])

            def do_v(t):
                # token-major V: stationary = xT chunk, moving = Wv rows.
                # Copy per-head-sub 64-col chunks into the 128-strided vo slots.
                for half in range(2):
                    ps = ps_a.tile([128, BLK], f32, name="psv", tag="psa")
                    for h in range(8):
                        nc.tensor.matmul(ps[:], sb_xT[h][:, 128 * t:128 * (t + 1)],
                                         sb_wv[h][:, 512 * half:512 * (half + 1)],
                                         start=(h == 0), stop=(h == 7))
                    s_ap, v_ap = ps[:], sb_vo[t][:]
                    src = bass.AP(tensor=s_ap.tensor, offset=s_ap.offset,
                                  ap=[s_ap.ap[0], [64, 8], [1, 64]])
                    dst = bass.AP(tensor=v_ap.tensor, offset=v_ap.offset + 128 * 8 * half,
                                  ap=[v_ap.ap[0], [128, 8], [1, 64]])
                    nc.vector.tensor_copy(dst, src)

            def do_scores(c, qbs=range(NQB)):
                # scores + exp + mask for heads 2c (sub=0), 2c+1 (sub=1)
                for qb in qbs:
                    e = att.tile([128, 768], bf16, name="exps", tag="exps")
                    for sub in range(2):
                        ps = ps_b.tile([128, 384], f32, name="pss", tag="psb")
                        for rel in range(3):
                            kc = qb + rel
                            nc.tensor.matmul(
                                ps[:, 128 * rel:128 * (rel + 1)],
                                sb_kt[c][64 * sub:64 * (sub + 1), 128 * kc:128 * (kc + 1)],
                                sb_qt[c][64 * sub:64 * (sub + 1), 128 * qb:128 * (qb + 1)])
                        nc.scalar.activation(e[:, 384 * sub:384 * (sub + 1)], ps[:], Exp)
                    # mask both heads, off-diagonal chunks only (rel=1 is
                    # always fully valid): strided per-chunk views
                    m = sb_mask[:, 384 * qb:384 * (qb + 1)]
                    for rel in (0, 2):
                        mv = bass.AP(tensor=m.tensor, offset=m.offset + 128 * rel,
                                     ap=[m.ap[0], [0, 2], [1, 128]])
                        ev = bass.AP(tensor=e.tensor, offset=e[:].offset + 128 * rel,
                                     ap=[e[:].ap[0], [384, 2], [1, 128]])
                        nc.gpsimd.tensor_mul(ev, ev, mv)
                    exps[(c, qb)] = e

            def do_pv(c):
                for sub in range(2):
                    j = 2 * c + sub
                    # after S7, ps_b is idle: borrow it for the tail pairs'
                    # second accumulator to deepen the PSUM ring.
                    if c >= 5 and sub == 1:
                        po = ps_b.tile([128, BLK], f32, name="po2", tag="psb")
                    else:
                        po = ps_po.tile([128, BLK], f32, name="po", tag="po")
                    for qb in range(NQB):
                        for rel in range(3):
                            kc = qb + rel
                            nc.tensor.matmul(
                                po[:, 128 * qb:128 * (qb + 1)],
                                sb_vo[kc][:, 128 * j:128 * (j + 1)],
                                exps[(c, qb)][:, 384 * sub + 128 * rel:384 * sub + 128 * (rel + 1)],
                                start=(rel == 0), stop=(rel == 2))
                    # denominator is replicated on partitions 64..127; the
                    # approx-fast reciprocal needs an SBUF fp32 source, so
                    # stage it through the scalar engine (denom >= 1: safe).
                    pr = rbp.tile([64, BLK], f32, name="pr", tag="pr")
                    nc.scalar.copy(pr[:], po[64:128, :])
                    rb = rbp.tile([64, BLK], f32, name="rb", tag="rb")
                    nc.vector.reciprocal_approx_fast(out=rb[:], in_=pr[:])
                    nc.vector.tensor_mul(sb_oc[c][64 * sub:64 * (sub + 1), :],
                                         po[0:64, :], rb[:])
                for qb in range(NQB):
                    del exps[(c, qb)]

            # ---- software-pipelined issue order -------------------------
            for c in range(8):
                do_q(c)
            do_v(0); do_v(1); do_k(0)
            do_v(2); do_k(1); do_scores(0)
            do_v(3); do_k(2); do_scores(1)
            do_v(4); do_k(3); do_scores(2)
            do_v(5); do_k(4); do_scores(3)
            do_k(5); do_scores(4)
            do_k(6); do_pv(0); do_scores(5)
            do_k(7); do_pv(1); do_scores(6, (0, 1))
            do_pv(2); do_scores(6, (2, 3))
            do_pv(3); do_scores(7, (0, 1))
            do_pv(4); do_scores(7, (2, 3))
            do_pv(5); do_pv(6)

            # open the first output block's accumulation before the last PV
            # so the final normalize chain hides behind matmuls.
            ps0 = ps_a.tile([128, BLK], f32, name="psy", tag="psa")
            for f in range(6):
                nc.tensor.matmul(ps0[:], sb_wo[f][:, 0:128], sb_oc[f][:],
                                 start=(f == 0), stop=False)
            do_pv(7)

            # ---- output projection --------------------------------------
            for c in range(8):
                ps = ps0 if c == 0 else ps_a.tile([128, BLK], f32, name="psy", tag="psa")
                for f in range(6 if c == 0 else 0, 8):
                    nc.tensor.matmul(ps[:], sb_wo[f][:, 128 * c:128 * (c + 1)],
                                     sb_oc[f][:], start=(f == 0), stop=(f == 7))
                yt = yout.tile([128, BLK], bf16, name="yt", tag="yt")
                nc.scalar.activation(yt[:], ps[:], Id, bias=sb_bo[:, c:c + 1])
                nc.sync.dma_start(d_out[128 * c:128 * (c + 1), :], yt[:])

    nc.compile()
    return nc


def _get_compiled():
    global _COMPILED
    if _COMPILED is None:
        _COMPILED = _build_bass()
    return _COMPILED


def kernel(x, Wq, bq, Wk, bk, Wv, bv, Wo, bo, _trace=False):
    from concourse.bass_utils import run_bass_kernel_spmd

    in_maps = _build_core_inputs(x, Wq, bq, Wk, bk, Wv, bv, Wo, bo)
    nc = _get_compiled()
    res = run_bass_kernel_spmd(nc, in_maps, core_ids=list(range(NCORES)),
                               trace=_trace)
    out = np.empty((B, S, H), np.float32)
    for c in range(NCORES):
        b, blk = divmod(c, 4)
        out[b, blk * BLK:(blk + 1) * BLK, :] = res.results[c]["out"].T.astype(np.float32)
    if _trace:
        return out, res
    return out
